# revision 15
# baseline (speedup 1.0000x reference)
import sys

sys.path.insert(0, "/opt/trn_rl_repo")

import numpy as np
import ml_dtypes

import concourse.bass as bass
import concourse.bacc as bacc
import concourse.tile as tile
from concourse.bass_utils import run_bass_kernel_spmd
from concourse import mybir

B, L, D, H = 2, 2048, 1024, 16
DH = 64          # dim per head
HPC = 4          # heads per core
CPC = HPC * DH   # feature cols per core = 256
NCORES = 8

MM_DT = "bfloat16"
NP_MM = ml_dtypes.bfloat16 if MM_DT == "bfloat16" else np.float32

_CACHE = {}


def build_nc(mm_dt: str):
    nc = bacc.Bacc()
    mm_dt = mybir.dt(mm_dt)
    fp32 = mybir.dt.float32

    # x tensors host-packed to the exact SBUF layout [p, lg, dc, c]:
    # value (p, lg, dc, c) = x.T[dc*128 + p, lg*512 + c].  Each DMA chunk
    # (one lg) is 8 KiB contiguous per partition -> 128 fat descriptors.
    xq = nc.declare_dram_parameter("xq", (128, 4, 8, 512), mm_dt, isOutput=False)
    xk = nc.declare_dram_parameter("xk", (128, 4, 8, 512), mm_dt, isOutput=False)
    xv = nc.declare_dram_parameter("xv", (128, 4, 8, 512), mm_dt, isOutput=False)
    # weights host-packed to SBUF layout [p, dc, f] = W[dc*128+p, f]
    wq = nc.declare_dram_parameter("wq", (128, 8, CPC), mm_dt, isOutput=False)
    wk = nc.declare_dram_parameter("wk", (128, 8, CPC), mm_dt, isOutput=False)
    wv = nc.declare_dram_parameter("wv", (128, 8, CPC), mm_dt, isOutput=False)
    wo = nc.declare_dram_parameter("wo", (CPC, D), mm_dt, isOutput=False)
    bq = nc.declare_dram_parameter("bq", (128, 2), fp32, isOutput=False)
    bk = nc.declare_dram_parameter("bk", (128, 2), fp32, isOutput=False)
    y = nc.declare_dram_parameter("y", (L, D), mm_dt, isOutput=True)   # partial out

    from contextlib import ExitStack

    with ExitStack() as es:
        tc = es.enter_context(tile.TileContext(nc))
        # NOTE: bufs are per named tag
        xt_pool = es.enter_context(tc.tile_pool(name="xt", bufs=1))     # 3 tags [128,4,8,512]
        w_pool = es.enter_context(tc.tile_pool(name="w", bufs=1))       # 3 tags [128,8,256]
        wo_pool = es.enter_context(tc.tile_pool(name="wo", bufs=1))     # 2 tags [128,1024]
        bias_pool = es.enter_context(tc.tile_pool(name="bias", bufs=1))
        qt_pool = es.enter_context(tc.tile_pool(name="qt", bufs=1))     # 2 tags [128,2048]
        kt_pool = es.enter_context(tc.tile_pool(name="kt", bufs=1))
        # partition-swapped duplicates (head at rows 0:64 <-> 64:128) so an
        # S pair's two kt-tile matmuls can run as concurrent PE row-tiles
        qt2_pool = es.enter_context(tc.tile_pool(name="qt2", bufs=1))
        kt2_pool = es.enter_context(tc.tile_pool(name="kt2", bufs=1))
        vn_pool = es.enter_context(tc.tile_pool(name="vn", bufs=1))     # [128,16,4,65]
        pt_pool = es.enter_context(tc.tile_pool(name="pt", bufs=6))     # [128,512]
        zr_pool = es.enter_context(tc.tile_pool(name="zr", bufs=3))     # [1,512]
        zbs_pool = es.enter_context(tc.tile_pool(name="zbs", bufs=3))   # [64,512]
        ot_pool = es.enter_context(tc.tile_pool(name="ot", bufs=1))     # 2 tags [128,2048]
        y_pool = es.enter_context(tc.tile_pool(name="ysb", bufs=4))     # [128,512]
        psA = es.enter_context(tc.tile_pool(name="psA", bufs=2, space="PSUM"))
        psS = es.enter_context(tc.tile_pool(name="psS", bufs=2, space="PSUM"))
        psOT = es.enter_context(tc.tile_pool(name="psOT", bufs=2, space="PSUM"))
        if True:
            # ---- load inputs over TWO hw dma queues (sync + gpsimd) ---------
            # A single dynamic queue sustains only ~146 GB/s; the 15 MB input
            # stream must ride two queues to keep pace with consumption.
            # Each queue is FIFO, so per-queue order == consumption order;
            # x chunks are split into partition halves (one per queue).
            wk_sb = w_pool.tile([128, 8, CPC], mm_dt, name="wk")
            bk_sb = bias_pool.tile([128, 2], fp32, name="bk")
            wq_sb = w_pool.tile([128, 8, CPC], mm_dt, name="wq")
            bq_sb = bias_pool.tile([128, 2], fp32, name="bq")
            wv_sb = w_pool.tile([128, 8, CPC], mm_dt, name="wv")
            xk_sb = xt_pool.tile([128, 4, 8, 512], mm_dt, name="xk")
            xq_sb = xt_pool.tile([128, 4, 8, 512], mm_dt, name="xq")
            xv_sb = xt_pool.tile([128, 4, 8, 512], mm_dt, name="xv")

            # single queue: the per-core DMA pool is shared, so one FIFO in
            # strict consumption order beats splitting bandwidth
            nc.sync.dma_start(out=wk_sb, in_=wk[:, :, :])
            nc.sync.dma_start(out=bk_sb, in_=bk[:, :])
            nc.sync.dma_start(out=xk_sb[:, 0], in_=xk[:, 0])
            nc.sync.dma_start(out=wq_sb, in_=wq[:, :, :])
            nc.sync.dma_start(out=bq_sb, in_=bq[:, :])
            nc.sync.dma_start(out=xq_sb[:, 0], in_=xq[:, 0])
            nc.sync.dma_start(out=wv_sb, in_=wv[:, :, :])
            nc.sync.dma_start(out=xv_sb[:, 0], in_=xv[:, 0])

            # qt chunks are needed at each group's START; kt/V chunks only at
            # its DIAG pairs (late in head 0) -- order the stream accordingly
            nc.sync.dma_start(out=xq_sb[:, 1], in_=xq[:, 1])
            nc.sync.dma_start(out=xk_sb[:, 1], in_=xk[:, 1])
            nc.sync.dma_start(out=xv_sb[:, 1], in_=xv[:, 1])
            nc.sync.dma_start(out=xq_sb[:, 2], in_=xq[:, 2])
            wo_sb = []
            for cc in range(2):
                t = wo_pool.tile([128, D], mm_dt, name=f"wo{cc}")
                nc.sync.dma_start(out=t, in_=wo[cc * 128:(cc + 1) * 128, :])
                wo_sb.append(t)
            nc.sync.dma_start(out=xk_sb[:, 2], in_=xk[:, 2])
            nc.sync.dma_start(out=xv_sb[:, 2], in_=xv[:, 2])
            nc.sync.dma_start(out=xq_sb[:, 3], in_=xq[:, 3])
            nc.sync.dma_start(out=xk_sb[:, 3], in_=xk[:, 3])
            nc.sync.dma_start(out=xv_sb[:, 3], in_=xv[:, 3])

            # ---- stage A helpers (emitted chunk-wise, interleaved with B) ---
            qt_sb = [qt_pool.tile([128, L], mm_dt, name=f"qt{i}") for i in range(2)]
            kt_sb = [kt_pool.tile([128, L], mm_dt, name=f"kt{i}") for i in range(2)]
            qt2_sb = [qt2_pool.tile([128, L], mm_dt, name=f"qt2_{i}") for i in range(2)]
            kt2_sb = [kt2_pool.tile([128, L], mm_dt, name=f"kt2_{i}") for i in range(2)]

            def emit_dup(dst2, srct, lg):
                # gpsimd SBUF->SBUF copies building the row-swapped duplicate
                sl = slice(lg * 512, (lg + 1) * 512)
                for cc in range(2):
                    nc.vector.tensor_copy(
                        out=dst2[cc][64:128, sl], in_=srct[cc][0:64, sl])
                    nc.vector.tensor_copy(
                        out=dst2[cc][0:64, sl], in_=srct[cc][64:128, sl])
            # V natural layout: [128(lt-part), 16 lt, 4 head, 65] (col 64 = ones)
            v_sb = vn_pool.tile([128, 16, 4, 65], mm_dt)
            nc.vector.memset(v_sb[:, :, :, 64:65], 1.0)

            def emit_QK_cc(dst, x_sb, w_sb, b_sb, lg, cc):
                ps = psA.tile([128, 512], fp32)
                for dc in range(8):
                    nc.tensor.matmul(
                        ps,
                        w_sb[:, dc, cc * 128:(cc + 1) * 128],
                        x_sb[:, lg, dc, :],
                        start=(dc == 0),
                        stop=(dc == 7),
                    )
                nc.vector.tensor_scalar_add(
                    out=dst[cc][:, lg * 512:(lg + 1) * 512],
                    in0=ps,
                    scalar1=b_sb[:, cc:cc + 1],
                )

            def emit_QK(dst, x_sb, w_sb, b_sb, lg):
                for cc in range(2):
                    emit_QK_cc(dst, x_sb, w_sb, b_sb, lg, cc)

            def emit_V(lt):
                ps = psA.tile([128, CPC], fp32)
                for dc in range(8):
                    nc.tensor.matmul(
                        ps,
                        xv_sb[:, lt // 4, dc, (lt % 4) * 128:(lt % 4 + 1) * 128],
                        wv_sb[:, dc, :],
                        start=(dc == 0),
                        stop=(dc == 7),
                    )
                nc.vector.tensor_copy(
                    out=v_sb[:, lt, :, 0:64],
                    in_=ps.rearrange("p (h d) -> p h d", d=64),
                )

            # prologue: just enough of A to start B(g4=0)
            emit_QK(kt_sb, xk_sb, wk_sb, bk_sb, 0)
            emit_dup(kt2_sb, kt_sb, 0)
            emit_QK(qt_sb, xq_sb, wq_sb, bq_sb, 0)
            emit_dup(qt2_sb, qt_sb, 0)
            for lt in range(4):
                emit_V(lt)

            # ---- stage B + C interleaved ------------------------------------
            ot_sb = [ot_pool.tile([128, L], mm_dt, name=f"ot{i}") for i in range(2)]
            y_view = y.rearrange("(lt p) c -> p lt c", p=128)

            def emit_C_lt(g4, li, copy_engs=("vector", "vector"),
                          dma_eng=None):
                # one seq tile (lt): both 512-col output halves into a single
                # [128,1024] staging tile, then ONE fat y DMA
                lt = g4 * 4 + li
                yt = y_pool.tile([128, 1024], mm_dt)
                for dg in range(2):
                    ps = psA.tile([128, 512], fp32)
                    for cc in range(2):
                        nc.tensor.matmul(
                            ps,
                            ot_sb[cc][:, lt * 128:(lt + 1) * 128],
                            wo_sb[cc][:, dg * 512:(dg + 1) * 512],
                            start=(cc == 0),
                            stop=(cc == 1),
                        )
                    dst = yt[:, dg * 512:(dg + 1) * 512]
                    if copy_engs[dg] == "scalar":
                        nc.scalar.activation(
                            out=dst, in_=ps,
                            func=mybir.ActivationFunctionType.Copy,
                            bias=0.0,
                        )
                    else:
                        nc.vector.tensor_copy(out=dst, in_=ps)
                (dma_eng or nc.sync).dma_start(
                    out=y_view[:, lt, :], in_=yt)

            def emit_C(g4):
                for li in range(4):
                    emit_C_lt(g4, li)

            # ---- filler queue: PE work the scheduler may pull into the
            # ACT-bound stretches of each attention pair loop ----------------
            pend = []

            def emit_fill(budget):
                while budget > 0 and pend:
                    cost, fn = pend.pop(0)
                    fn()
                    budget -= cost

            def flush_pend():
                while pend:
                    pend.pop(0)[1]()

            def populate_pend(g4):
                nx = g4 + 1
                items = []
                if g4 < 3:
                    items += [
                        (1700, lambda: emit_QK_cc(qt_sb, xq_sb, wq_sb, bq_sb, nx, 0)),
                        (1700, lambda: emit_QK_cc(qt_sb, xq_sb, wq_sb, bq_sb, nx, 1)),
                        (0, lambda: emit_dup(qt2_sb, qt_sb, nx)),
                    ]
                # all three completed C groups ride into g4=3: its exp-bound
                # stretch needs more PE filler than its own work provides,
                # while g4=2 retains a PE surplus without C(0)
                if g4 == 3:
                    for cg in (0, 1, 2):
                        items += [(850, lambda cg=cg, li=li: emit_C_lt(cg, li))
                                  for li in range(4)]
                pend[:] = items

            for g4 in range(4):
                populate_pend(g4)
                # kt chunk g4 and V tiles 4g4..4g4+3 are first consumed by
                # head 0's diagonal pairs: produce them inside head 0's loop
                # (late DMA deadline + PE filler for the ACT-bound stretch)
                pre_diag = []
                if g4 >= 1:
                    pre_diag = [
                        lambda: emit_QK_cc(kt_sb, xk_sb, wk_sb, bk_sb, g4, 0),
                        lambda: emit_QK_cc(kt_sb, xk_sb, wk_sb, bk_sb, g4, 1),
                        lambda: emit_dup(kt2_sb, kt_sb, g4),
                    ] + [
                        (lambda lt=4 * g4 + i: emit_V(lt)) for i in range(4)
                    ]
                for h in range(HPC):
                    cc = h // 2
                    ro = (h % 2) * 64
                    nkt = g4 * 4 + 4
                    ot_ps = psOT.tile([65, 512], fp32)
                    pts = {}

                    def emit_S_pair(k0):
                        # two kt tiles share a [128,1024] PSUM pair; non-diag
                        # pairs get a single wide exp (saves ACT overhead)
                        diag = (k0 // 4 == g4)
                        st = psS.tile([128, 1024], fp32, name="st2")
                        ro2 = 64 - ro
                        for j in range(2):
                            kt = k0 + j
                            off = 128 * (kt % 4) if diag else 0
                            base = j * 512
                            if j == 0:
                                lhs = kt_sb[cc][ro:ro + 64,
                                                kt * 128:(kt + 1) * 128]
                                rhs = qt_sb[cc][ro:ro + 64,
                                                g4 * 512 + off:(g4 + 1) * 512]
                            else:
                                # duplicate at the flipped partition offset ->
                                # different PE row group -> runs concurrently
                                lhs = kt2_sb[cc][ro2:ro2 + 64,
                                                 kt * 128:(kt + 1) * 128]
                                rhs = qt2_sb[cc][ro2:ro2 + 64,
                                                 g4 * 512 + off:(g4 + 1) * 512]
                            nc.tensor.matmul(
                                st[:, base + off:base + 512],
                                lhs,
                                rhs,
                                start=True,
                                stop=True,
                            )
                        pt = pt_pool.tile([128, 1024], mm_dt, name="pt2")
                        if not diag:
                            nc.scalar.activation(
                                out=pt,
                                in_=st,
                                func=mybir.ActivationFunctionType.Exp,
                                scale=0.125,
                            )
                        else:
                            for j in range(2):
                                kt = k0 + j
                                off = 128 * (kt % 4)
                                base = j * 512
                                nc.scalar.activation(
                                    out=pt[:, base + off:base + 512],
                                    in_=st[:, base + off:base + 512],
                                    func=mybir.ActivationFunctionType.Exp,
                                    scale=0.125,
                                )
                                # keep iff f - p - off >= 0. Cols >= off+128
                                # all-keep (skip); cols < off all-fill (zeroes
                                # the stale region the partial exp skipped).
                                w = off + 128
                                nc.gpsimd.affine_select(
                                    out=pt[:, base:base + w],
                                    in_=pt[:, base:base + w],
                                    compare_op=mybir.AluOpType.is_ge,
                                    fill=0.0,
                                    base=-off,
                                    channel_multiplier=-1,
                                    pattern=[[1, w]],
                                )
                        pts[k0] = pt[:, 0:512]
                        pts[k0 + 1] = pt[:, 512:1024]

                    def emit_P(kt):
                        nc.tensor.matmul(
                            ot_ps,
                            v_sb[:, kt, h, :],
                            pts.pop(kt),
                            start=(kt == 0),
                            stop=(kt == nkt - 1),
                        )

                    npair = nkt // 2
                    for kp in range(npair):
                        if pre_diag and kp == 2 * g4 - 1:
                            # one pair of slack before the diag S needs kt(g4)
                            for fn in pre_diag:
                                fn()
                            pre_diag = []
                        emit_S_pair(2 * kp)
                        if kp >= 1:
                            emit_P(2 * kp - 2)
                            emit_P(2 * kp - 1)
                            emit_fill(700)
                    emit_P(nkt - 2)
                    emit_P(nkt - 1)
                    emit_fill(700)

                    if g4 == 3 and h == 3:
                        # the very last divide chain has nothing left to hide
                        # behind: run leftover C fillers on the PE while the
                        # serial DVE/gpsimd chain drains
                        flush_pend()
                    # divide by Z (row 64) -- off the PE entirely.
                    # NB: reciprocal_approx_fast reading PSUM directly is
                    # silently wrong; bounce the row through SBUF first.
                    zrow = zr_pool.tile([1, 512], fp32, name="zrow")
                    nc.vector.tensor_copy(out=zrow, in_=ot_ps[64:65, :])
                    zr = zr_pool.tile([1, 512], fp32, name="zr")
                    nc.vector.reciprocal_approx_fast(out=zr, in_=zrow)
                    zb = zbs_pool.tile([64, 512], fp32)
                    nc.gpsimd.partition_broadcast(out_ap=zb, in_ap=zr)
                    nc.vector.tensor_mul(
                        out=ot_sb[cc][ro:ro + 64, g4 * 512:(g4 + 1) * 512],
                        in0=ot_ps[0:64, :],
                        in1=zb,
                    )
                    # leftover fillers for this head slot
                    emit_fill(1600)
                flush_pend()
            # final C group: copies split over DVE+ACT, y DMAs over the
            # now-idle vector/scalar engine queues (descgen in parallel)
            for li in range(4):
                emit_C_lt(3, li, copy_engs=("vector", "scalar"),
                          dma_eng=(nc.vector if li % 2 == 0 else nc.scalar))

    nc.compile()
    return nc


def _get_nc(mm_dt: str):
    if mm_dt not in _CACHE:
        _CACHE[mm_dt] = build_nc(mm_dt)
    return _CACHE[mm_dt]


def _pack_x(xb):
    # [L, D] fp32 -> [128, 4, 8, 512] bf16 with (p, lg, dc, c) = xT[dc*128+p, lg*512+c]
    xt = np.ascontiguousarray(xb.T).astype(NP_MM)          # [D, L]
    return np.ascontiguousarray(
        xt.reshape(8, 128, 4, 512).transpose(1, 2, 0, 3)
    )


def _pack_w(Wslice):
    # [D, CPC] -> [128, 8, CPC] with (p, dc, f) = W[dc*128+p, f]
    return np.ascontiguousarray(
        Wslice.astype(NP_MM).reshape(8, 128, CPC).transpose(1, 0, 2)
    )


def _pack_b(bslice):
    # [CPC] -> [128, 2] with (p, cc) = b[cc*128+p]
    return np.ascontiguousarray(bslice.reshape(2, 128).T).astype(np.float32)


def kernel(q, k, v, mask, Wq, bq, Wk, bk, Wv, bv, Wo, bo, _trace=False):
    nc = _get_nc(MM_DT)

    xq_p = [_pack_x(q[b]) for b in range(B)]
    xk_p = [_pack_x(k[b]) for b in range(B)]
    xv_p = [_pack_x(v[b]) for b in range(B)]

    in_maps = []
    for c in range(NCORES):
        b = c // 4
        g = c % 4
        s = slice(g * CPC, (g + 1) * CPC)
        in_maps.append({
            "xq": xq_p[b],
            "xk": xk_p[b],
            "xv": xv_p[b],
            "wq": _pack_w(Wq[:, s]),
            "wk": _pack_w(Wk[:, s]),
            "wv": _pack_w(Wv[:, s]),
            "wo": np.ascontiguousarray(Wo[s, :]).astype(NP_MM),
            "bq": _pack_b(bq[s]),
            "bk": _pack_b(bk[s]),
        })

    res = run_bass_kernel_spmd(nc, in_maps, list(range(NCORES)), trace=_trace)

    # host gather: out[b] = sum_g y_core(b,g) + (bo + bv @ Wo)
    const = (bo + bv.astype(np.float64) @ Wo.astype(np.float64)).astype(np.float64)
    out = np.zeros((B, L, D), np.float64)
    for c in range(NCORES):
        out[c // 4] += res.results[c]["y"].astype(np.float64)
    out += const[None, None, :]
    kernel.last_exec_time_ns = res.exec_time_ns
    return out.astype(np.float32)


# revision 17
# speedup vs baseline: 1.0124x; 1.0124x over previous
import sys

sys.path.insert(0, "/opt/trn_rl_repo")

import numpy as np
import ml_dtypes

import concourse.bass as bass
import concourse.bacc as bacc
import concourse.tile as tile
from concourse.bass_utils import run_bass_kernel_spmd
from concourse import mybir

B, L, D, H = 2, 2048, 1024, 16
DH = 64          # dim per head
HPC = 4          # heads per core
CPC = HPC * DH   # feature cols per core = 256
NCORES = 8

MM_DT = "bfloat16"
NP_MM = ml_dtypes.bfloat16 if MM_DT == "bfloat16" else np.float32

_CACHE = {}


def build_nc(mm_dt: str):
    nc = bacc.Bacc()
    mm_dt = mybir.dt(mm_dt)
    fp32 = mybir.dt.float32

    # x tensors host-packed to the exact SBUF layout [p, lg, dc, c]:
    # value (p, lg, dc, c) = x.T[dc*128 + p, lg*512 + c].  Each DMA chunk
    # (one lg) is 8 KiB contiguous per partition -> 128 fat descriptors.
    xq = nc.declare_dram_parameter("xq", (128, 4, 8, 512), mm_dt, isOutput=False)
    xk = nc.declare_dram_parameter("xk", (128, 4, 8, 512), mm_dt, isOutput=False)
    xv = nc.declare_dram_parameter("xv", (128, 4, 8, 512), mm_dt, isOutput=False)
    # weights host-packed to SBUF layout [p, dc, f] = W[dc*128+p, f]
    wq = nc.declare_dram_parameter("wq", (128, 8, CPC), mm_dt, isOutput=False)
    wk = nc.declare_dram_parameter("wk", (128, 8, CPC), mm_dt, isOutput=False)
    wv = nc.declare_dram_parameter("wv", (128, 8, CPC), mm_dt, isOutput=False)
    wo = nc.declare_dram_parameter("wo", (CPC, D), mm_dt, isOutput=False)
    bq = nc.declare_dram_parameter("bq", (128, 2), fp32, isOutput=False)
    bk = nc.declare_dram_parameter("bk", (128, 2), fp32, isOutput=False)
    y = nc.declare_dram_parameter("y", (L, D), mm_dt, isOutput=True)   # partial out

    from contextlib import ExitStack

    with ExitStack() as es:
        tc = es.enter_context(tile.TileContext(nc))
        # NOTE: bufs are per named tag
        xt_pool = es.enter_context(tc.tile_pool(name="xt", bufs=1))     # 3 tags [128,4,8,512]
        w_pool = es.enter_context(tc.tile_pool(name="w", bufs=1))       # 3 tags [128,8,256]
        wo_pool = es.enter_context(tc.tile_pool(name="wo", bufs=1))     # 2 tags [128,1024]
        bias_pool = es.enter_context(tc.tile_pool(name="bias", bufs=1))
        qt_pool = es.enter_context(tc.tile_pool(name="qt", bufs=1))     # 2 tags [128,2048]
        kt_pool = es.enter_context(tc.tile_pool(name="kt", bufs=1))
        # partition-swapped duplicates (head at rows 0:64 <-> 64:128) so an
        # S pair's two kt-tile matmuls can run as concurrent PE row-tiles
        qt2_pool = es.enter_context(tc.tile_pool(name="qt2", bufs=1))
        kt2_pool = es.enter_context(tc.tile_pool(name="kt2", bufs=1))
        vn_pool = es.enter_context(tc.tile_pool(name="vn", bufs=1))     # [128,16,4,65]
        pt_pool = es.enter_context(tc.tile_pool(name="pt", bufs=6))     # [128,512]
        zr_pool = es.enter_context(tc.tile_pool(name="zr", bufs=3))     # [1,512]
        zbs_pool = es.enter_context(tc.tile_pool(name="zbs", bufs=3))   # [64,512]
        ot_pool = es.enter_context(tc.tile_pool(name="ot", bufs=1))     # 2 tags [128,2048]
        y_pool = es.enter_context(tc.tile_pool(name="ysb", bufs=4))     # [128,512]
        psA = es.enter_context(tc.tile_pool(name="psA", bufs=2, space="PSUM"))
        psS = es.enter_context(tc.tile_pool(name="psS", bufs=2, space="PSUM"))
        psOT = es.enter_context(tc.tile_pool(name="psOT", bufs=2, space="PSUM"))
        if True:
            # ---- load inputs over TWO hw dma queues (sync + gpsimd) ---------
            # A single dynamic queue sustains only ~146 GB/s; the 15 MB input
            # stream must ride two queues to keep pace with consumption.
            # Each queue is FIFO, so per-queue order == consumption order;
            # x chunks are split into partition halves (one per queue).
            wk_sb = w_pool.tile([128, 8, CPC], mm_dt, name="wk")
            bk_sb = bias_pool.tile([128, 2], fp32, name="bk")
            wq_sb = w_pool.tile([128, 8, CPC], mm_dt, name="wq")
            bq_sb = bias_pool.tile([128, 2], fp32, name="bq")
            wv_sb = w_pool.tile([128, 8, CPC], mm_dt, name="wv")
            xk_sb = xt_pool.tile([128, 4, 8, 512], mm_dt, name="xk")
            xq_sb = xt_pool.tile([128, 4, 8, 512], mm_dt, name="xq")
            xv_sb = xt_pool.tile([128, 4, 8, 512], mm_dt, name="xv")

            # single queue: the per-core DMA pool is shared, so one FIFO in
            # strict consumption order beats splitting bandwidth
            nc.sync.dma_start(out=wk_sb, in_=wk[:, :, :])
            nc.sync.dma_start(out=bk_sb, in_=bk[:, :])
            nc.sync.dma_start(out=xk_sb[:, 0], in_=xk[:, 0])
            nc.sync.dma_start(out=wq_sb, in_=wq[:, :, :])
            nc.sync.dma_start(out=bq_sb, in_=bq[:, :])
            nc.sync.dma_start(out=xq_sb[:, 0], in_=xq[:, 0])
            nc.sync.dma_start(out=wv_sb, in_=wv[:, :, :])
            nc.sync.dma_start(out=xv_sb[:, 0], in_=xv[:, 0])

            # qt chunks are needed at each group's START; kt/V chunks only at
            # its DIAG pairs (late in head 0) -- order the stream accordingly
            nc.sync.dma_start(out=xq_sb[:, 1], in_=xq[:, 1])
            nc.sync.dma_start(out=xk_sb[:, 1], in_=xk[:, 1])
            nc.sync.dma_start(out=xv_sb[:, 1], in_=xv[:, 1])
            nc.sync.dma_start(out=xq_sb[:, 2], in_=xq[:, 2])
            wo_sb = []
            for cc in range(2):
                t = wo_pool.tile([128, D], mm_dt, name=f"wo{cc}")
                nc.sync.dma_start(out=t, in_=wo[cc * 128:(cc + 1) * 128, :])
                wo_sb.append(t)
            nc.sync.dma_start(out=xk_sb[:, 2], in_=xk[:, 2])
            nc.sync.dma_start(out=xv_sb[:, 2], in_=xv[:, 2])
            nc.sync.dma_start(out=xq_sb[:, 3], in_=xq[:, 3])
            nc.sync.dma_start(out=xk_sb[:, 3], in_=xk[:, 3])
            nc.sync.dma_start(out=xv_sb[:, 3], in_=xv[:, 3])

            # ---- stage A helpers (emitted chunk-wise, interleaved with B) ---
            qt_sb = [qt_pool.tile([128, L], mm_dt, name=f"qt{i}") for i in range(2)]
            kt_sb = [kt_pool.tile([128, L], mm_dt, name=f"kt{i}") for i in range(2)]
            qt2_sb = [qt2_pool.tile([128, L], mm_dt, name=f"qt2_{i}") for i in range(2)]
            kt2_sb = [kt2_pool.tile([128, L], mm_dt, name=f"kt2_{i}") for i in range(2)]

            def emit_dup(dst2, srct, lg):
                # gpsimd SBUF->SBUF copies building the row-swapped duplicate
                sl = slice(lg * 512, (lg + 1) * 512)
                for cc in range(2):
                    nc.vector.tensor_copy(
                        out=dst2[cc][64:128, sl], in_=srct[cc][0:64, sl])
                    nc.vector.tensor_copy(
                        out=dst2[cc][0:64, sl], in_=srct[cc][64:128, sl])
            # V natural layout: [128(lt-part), 16 lt, 4 head, 65] (col 64 = ones)
            v_sb = vn_pool.tile([128, 16, 4, 65], mm_dt)
            nc.vector.memset(v_sb[:, :, :, 64:65], 1.0)

            def emit_QK_cc(dst, x_sb, w_sb, b_sb, lg, cc):
                ps = psA.tile([128, 512], fp32)
                for dc in range(8):
                    nc.tensor.matmul(
                        ps,
                        w_sb[:, dc, cc * 128:(cc + 1) * 128],
                        x_sb[:, lg, dc, :],
                        start=(dc == 0),
                        stop=(dc == 7),
                    )
                nc.vector.tensor_scalar_add(
                    out=dst[cc][:, lg * 512:(lg + 1) * 512],
                    in0=ps,
                    scalar1=b_sb[:, cc:cc + 1],
                )

            def emit_QK(dst, x_sb, w_sb, b_sb, lg):
                for cc in range(2):
                    emit_QK_cc(dst, x_sb, w_sb, b_sb, lg, cc)

            def emit_V(lt):
                ps = psA.tile([128, CPC], fp32)
                for dc in range(8):
                    nc.tensor.matmul(
                        ps,
                        xv_sb[:, lt // 4, dc, (lt % 4) * 128:(lt % 4 + 1) * 128],
                        wv_sb[:, dc, :],
                        start=(dc == 0),
                        stop=(dc == 7),
                    )
                nc.vector.tensor_copy(
                    out=v_sb[:, lt, :, 0:64],
                    in_=ps.rearrange("p (h d) -> p h d", d=64),
                )

            # prologue: just enough of A to start B(g4=0)
            emit_QK(kt_sb, xk_sb, wk_sb, bk_sb, 0)
            emit_dup(kt2_sb, kt_sb, 0)
            emit_QK(qt_sb, xq_sb, wq_sb, bq_sb, 0)
            emit_dup(qt2_sb, qt_sb, 0)
            for lt in range(4):
                emit_V(lt)

            # ---- stage B + C interleaved ------------------------------------
            ot_sb = [ot_pool.tile([128, L], mm_dt, name=f"ot{i}") for i in range(2)]
            y_view = y.rearrange("(lt p) c -> p lt c", p=128)

            def emit_C_lt(g4, li, copy_engs=("vector", "vector"),
                          dma_eng=None):
                # one seq tile (lt): both 512-col output halves into a single
                # [128,1024] staging tile, then ONE fat y DMA
                lt = g4 * 4 + li
                yt = y_pool.tile([128, 1024], mm_dt)
                for dg in range(2):
                    ps = psA.tile([128, 512], fp32)
                    for cc in range(2):
                        nc.tensor.matmul(
                            ps,
                            ot_sb[cc][:, lt * 128:(lt + 1) * 128],
                            wo_sb[cc][:, dg * 512:(dg + 1) * 512],
                            start=(cc == 0),
                            stop=(cc == 1),
                        )
                    dst = yt[:, dg * 512:(dg + 1) * 512]
                    if copy_engs[dg] == "scalar":
                        nc.scalar.activation(
                            out=dst, in_=ps,
                            func=mybir.ActivationFunctionType.Copy,
                            bias=0.0,
                        )
                    else:
                        nc.vector.tensor_copy(out=dst, in_=ps)
                (dma_eng or nc.sync).dma_start(
                    out=y_view[:, lt, :], in_=yt)

            def emit_C(g4):
                for li in range(4):
                    emit_C_lt(g4, li)

            # ---- filler queue: PE work the scheduler may pull into the
            # ACT-bound stretches of each attention pair loop ----------------
            pend = []

            def emit_fill(budget):
                while budget > 0 and pend:
                    cost, fn = pend.pop(0)
                    fn()
                    budget -= cost

            def flush_pend():
                while pend:
                    pend.pop(0)[1]()

            def populate_pend(g4):
                nx = g4 + 1
                items = []
                if g4 < 3:
                    items += [
                        (1700, lambda: emit_QK_cc(qt_sb, xq_sb, wq_sb, bq_sb, nx, 0)),
                        (1700, lambda: emit_QK_cc(qt_sb, xq_sb, wq_sb, bq_sb, nx, 1)),
                        (0, lambda: emit_dup(qt2_sb, qt_sb, nx)),
                    ]
                # all completed C groups ride into g4=3: rationed pops cover
                # each head's exp-bound stretch, and the remainder flushes as
                # PE filler under the final serial divide chain
                if g4 == 3:
                    for cg in (0, 1, 2):
                        items += [(850, lambda cg=cg, li=li: emit_C_lt(cg, li))
                                  for li in range(4)]
                pend[:] = items

            for g4 in range(4):
                populate_pend(g4)
                # kt chunk g4 and V tiles 4g4..4g4+3 are first consumed by
                # head 0's diagonal pairs: produce them inside head 0's loop
                # (late DMA deadline + PE filler for the ACT-bound stretch)
                pre_diag = []
                if g4 >= 1:
                    pre_diag = [
                        lambda: emit_QK_cc(kt_sb, xk_sb, wk_sb, bk_sb, g4, 0),
                        lambda: emit_QK_cc(kt_sb, xk_sb, wk_sb, bk_sb, g4, 1),
                        lambda: emit_dup(kt2_sb, kt_sb, g4),
                    ] + [
                        (lambda lt=4 * g4 + i: emit_V(lt)) for i in range(4)
                    ]
                for h in range(HPC):
                    cc = h // 2
                    ro = (h % 2) * 64
                    nkt = g4 * 4 + 4
                    ot_ps = psOT.tile([65, 512], fp32)
                    pts = {}

                    def emit_S_pair(k0):
                        # two kt tiles share a [128,1024] PSUM pair; non-diag
                        # pairs get a single wide exp (saves ACT overhead)
                        diag = (k0 // 4 == g4)
                        st = psS.tile([128, 1024], fp32, name="st2")
                        ro2 = 64 - ro
                        for j in range(2):
                            kt = k0 + j
                            off = 128 * (kt % 4) if diag else 0
                            base = j * 512
                            if j == 0:
                                lhs = kt_sb[cc][ro:ro + 64,
                                                kt * 128:(kt + 1) * 128]
                                rhs = qt_sb[cc][ro:ro + 64,
                                                g4 * 512 + off:(g4 + 1) * 512]
                            else:
                                # duplicate at the flipped partition offset ->
                                # different PE row group -> runs concurrently
                                lhs = kt2_sb[cc][ro2:ro2 + 64,
                                                 kt * 128:(kt + 1) * 128]
                                rhs = qt2_sb[cc][ro2:ro2 + 64,
                                                 g4 * 512 + off:(g4 + 1) * 512]
                            nc.tensor.matmul(
                                st[:, base + off:base + 512],
                                lhs,
                                rhs,
                                start=True,
                                stop=True,
                            )
                        pt = pt_pool.tile([128, 1024], mm_dt, name="pt2")
                        if not diag:
                            nc.scalar.activation(
                                out=pt,
                                in_=st,
                                func=mybir.ActivationFunctionType.Exp,
                                scale=0.125,
                            )
                        else:
                            for j in range(2):
                                kt = k0 + j
                                off = 128 * (kt % 4)
                                base = j * 512
                                nc.scalar.activation(
                                    out=pt[:, base + off:base + 512],
                                    in_=st[:, base + off:base + 512],
                                    func=mybir.ActivationFunctionType.Exp,
                                    scale=0.125,
                                )
                                # keep iff f - p - off >= 0. Cols >= off+128
                                # all-keep (skip); cols < off all-fill (zeroes
                                # the stale region the partial exp skipped).
                                w = off + 128
                                nc.gpsimd.affine_select(
                                    out=pt[:, base:base + w],
                                    in_=pt[:, base:base + w],
                                    compare_op=mybir.AluOpType.is_ge,
                                    fill=0.0,
                                    base=-off,
                                    channel_multiplier=-1,
                                    pattern=[[1, w]],
                                )
                        pts[k0] = pt[:, 0:512]
                        pts[k0 + 1] = pt[:, 512:1024]

                    def emit_P(kt):
                        nc.tensor.matmul(
                            ot_ps,
                            v_sb[:, kt, h, :],
                            pts.pop(kt),
                            start=(kt == 0),
                            stop=(kt == nkt - 1),
                        )

                    npair = nkt // 2
                    for kp in range(npair):
                        if pre_diag and kp == 2 * g4 - 1:
                            # one pair of slack before the diag S needs kt(g4)
                            for fn in pre_diag:
                                fn()
                            pre_diag = []
                        emit_S_pair(2 * kp)
                        if kp >= 1:
                            emit_P(2 * kp - 2)
                            emit_P(2 * kp - 1)
                            emit_fill(150 if g4 == 3 else 700)
                    emit_P(nkt - 2)
                    emit_P(nkt - 1)
                    emit_fill(150 if g4 == 3 else 700)

                    if g4 == 3 and h == 3:
                        # leftover C fillers keep the PE busy while the final
                        # divide chain (DVE/gpsimd, ~3.8us serial) drains
                        flush_pend()
                    # divide by Z (row 64) -- off the PE entirely.
                    # NB: reciprocal_approx_fast reading PSUM directly is
                    # silently wrong; bounce the row through SBUF first.
                    zrow = zr_pool.tile([1, 512], fp32, name="zrow")
                    nc.vector.tensor_copy(out=zrow, in_=ot_ps[64:65, :])
                    zr = zr_pool.tile([1, 512], fp32, name="zr")
                    nc.vector.reciprocal_approx_fast(out=zr, in_=zrow)
                    zb = zbs_pool.tile([64, 512], fp32)
                    nc.gpsimd.partition_broadcast(out_ap=zb, in_ap=zr)
                    nc.vector.tensor_mul(
                        out=ot_sb[cc][ro:ro + 64, g4 * 512:(g4 + 1) * 512],
                        in0=ot_ps[0:64, :],
                        in1=zb,
                    )
                    # leftover fillers for this head slot
                    emit_fill(450 if g4 == 3 else 1600)
                flush_pend()
            # final C group: copies split over DVE+ACT, y DMAs over the
            # now-idle vector/scalar engine queues (descgen in parallel)
            for li in range(4):
                emit_C_lt(3, li, copy_engs=("vector", "scalar"),
                          dma_eng=(nc.vector if li % 2 == 0 else nc.scalar))

    nc.compile()
    return nc


def _get_nc(mm_dt: str):
    if mm_dt not in _CACHE:
        _CACHE[mm_dt] = build_nc(mm_dt)
    return _CACHE[mm_dt]


def _pack_x(xb):
    # [L, D] fp32 -> [128, 4, 8, 512] bf16 with (p, lg, dc, c) = xT[dc*128+p, lg*512+c]
    xt = np.ascontiguousarray(xb.T).astype(NP_MM)          # [D, L]
    return np.ascontiguousarray(
        xt.reshape(8, 128, 4, 512).transpose(1, 2, 0, 3)
    )


def _pack_w(Wslice):
    # [D, CPC] -> [128, 8, CPC] with (p, dc, f) = W[dc*128+p, f]
    return np.ascontiguousarray(
        Wslice.astype(NP_MM).reshape(8, 128, CPC).transpose(1, 0, 2)
    )


def _pack_b(bslice):
    # [CPC] -> [128, 2] with (p, cc) = b[cc*128+p]
    return np.ascontiguousarray(bslice.reshape(2, 128).T).astype(np.float32)


def kernel(q, k, v, mask, Wq, bq, Wk, bk, Wv, bv, Wo, bo, _trace=False):
    nc = _get_nc(MM_DT)

    xq_p = [_pack_x(q[b]) for b in range(B)]
    xk_p = [_pack_x(k[b]) for b in range(B)]
    xv_p = [_pack_x(v[b]) for b in range(B)]

    in_maps = []
    for c in range(NCORES):
        b = c // 4
        g = c % 4
        s = slice(g * CPC, (g + 1) * CPC)
        in_maps.append({
            "xq": xq_p[b],
            "xk": xk_p[b],
            "xv": xv_p[b],
            "wq": _pack_w(Wq[:, s]),
            "wk": _pack_w(Wk[:, s]),
            "wv": _pack_w(Wv[:, s]),
            "wo": np.ascontiguousarray(Wo[s, :]).astype(NP_MM),
            "bq": _pack_b(bq[s]),
            "bk": _pack_b(bk[s]),
        })

    res = run_bass_kernel_spmd(nc, in_maps, list(range(NCORES)), trace=_trace)

    # host gather: out[b] = sum_g y_core(b,g) + (bo + bv @ Wo)
    const = (bo + bv.astype(np.float64) @ Wo.astype(np.float64)).astype(np.float64)
    out = np.zeros((B, L, D), np.float64)
    for c in range(NCORES):
        out[c // 4] += res.results[c]["y"].astype(np.float64)
    out += const[None, None, :]
    kernel.last_exec_time_ns = res.exec_time_ns
    return out.astype(np.float32)
